# revision 51
# baseline (speedup 1.0000x reference)
"""Trainium2 Bass kernel for a dense transformer block (pre-LN, causal MHA + MLP).

Problem: x[64,256,384], 6 heads x 64, d_ff=1536.
Strategy: pure data parallel over batch -- each of 8 NeuronCores processes 8
batches with replicated weights; no collectives.

Design (vs the fp32r baseline, ~2.5x faster in CoreSim):
  * fp8(e4m3) weights pre-cast on the host; every QKV / W1 / W2 GEMM pass
    runs fp8 DoubleRow at 0.5 cyc/row -- K=384 contractions are padded to
    512 with zero weight rows so both passes pair 2 K-rows per PE cell.
  * key-major attention: scores computed directly as S^T[s,t] = kT.T @ qT
    (causal (s1,t0) block never computed; triangular mask accumulated onto
    the scores by a PE matmul so no cross-engine hop before exp). exp()
    output is already the AV matmul's lhsT layout -- no wei transposes.
  * softmax denominators ride along as a ones-column appended to V (column
    64 of each 66-wide head group); normalization happens token-major in
    one strided DVE multiply per token tile, after the AV matmul.
  * h / attn transposes ride the idle DMA engines (dma_start_transpose,
    bf16) with fp8 conversion on idle GpSimd; LayerNorm scale/offset and
    the Newton rsqrt also live on GpSimd.  PE / ACT(exp,relu) / DVE(LN
    stats, PSUM evacuations) end up near-balanced at ~63 us busy each.
  * PSUM is split into four 2-bank pools, one per GEMM phase (qk+v /
    scores / AV+Wo / MLP) -- a shared pool's buffer rotation otherwise
    serializes independent phases and halves throughput.
  * pipeline ramp/drain use latency-optimized variants (PE-transposes,
    ACT/DVE-split relus and qkT evacuations) since those engines idle there.
"""
import numpy as np
import ml_dtypes
from contextlib import ExitStack

import concourse.bass as bass
import concourse.tile as tile
from concourse import bacc, mybir
from concourse._compat import with_exitstack
from concourse.bass_utils import run_bass_kernel_spmd

F32 = mybir.dt.float32
BF16 = mybir.dt.bfloat16
F8 = mybir.dt.float8e4
AF = mybir.ActivationFunctionType
ALU = mybir.AluOpType
DR = mybir.MatmulPerfMode.DoubleRow

NPF8 = ml_dtypes.float8_e4m3
NPBF16 = ml_dtypes.bfloat16

N_CORES = 8
B, T, C = 64, 256, 384
H, HS = 6, 64
DFF = 4 * C
EPS = 1e-5
BL = B // N_CORES          # 8 batches per core
NT = T // 128              # 2 token-tiles per batch
KC = C // 128              # 3 feature tiles
KF = DFF // 128            # 12 ff tiles
GA = 66                    # attn head group stride (64 attn + d + pad)


@with_exitstack
def block_kernel(ctx: ExitStack, tc: tile.TileContext, flags: dict, repeat: int = 1):
    nc = tc.nc
    x_d = nc.dram_tensor("x", [BL, T, C], F32, kind="ExternalInput").ap()
    wqk_A_d = nc.dram_tensor("wqk_A", [128, 4 * C], F8, kind="ExternalInput").ap()
    wqk_B_d = nc.dram_tensor("wqk_B", [128, 4 * C], F8, kind="ExternalInput").ap()
    wv_A_d = nc.dram_tensor("wv_A", [128, 2 * C], F8, kind="ExternalInput").ap()
    wv_B_d = nc.dram_tensor("wv_B", [128, 2 * C], F8, kind="ExternalInput").ap()
    wo_d = nc.dram_tensor("wo_t", [128, KC * C], BF16, kind="ExternalInput").ap()
    w1_A_d = nc.dram_tensor("w1_A", [128, 2 * DFF], F8, kind="ExternalInput").ap()
    w1_B_d = nc.dram_tensor("w1_B", [128, 2 * DFF], F8, kind="ExternalInput").ap()
    w2_dr_d = nc.dram_tensor("w2_dr", [128, (KF // 2) * 2 * C], F8,
                             kind="ExternalInput").ap()
    b1_d = (nc.dram_tensor("b1", [DFF], F32, kind="ExternalInput").ap()
            if flags["b1"] else None)
    out_d = nc.dram_tensor("out", [BL, T, C], F32, kind="ExternalOutput").ap()
    opt = {}
    for nm in ["bo", "b2", "g1", "beta1", "g2", "beta2"]:
        if flags[nm]:
            opt[nm] = nc.dram_tensor(nm, [C], F32, kind="ExternalInput").ap()

    const = ctx.enter_context(tc.tile_pool(name="const", bufs=1))
    wp = ctx.enter_context(tc.tile_pool(name="wp", bufs=1))
    sb = ctx.enter_context(tc.tile_pool(name="sb", bufs=1))
    ps = ctx.enter_context(tc.tile_pool(name="ps", bufs=1, space="PSUM"))

    # ---------------- constants ----------------
    ident_bf = const.tile([128, 128], BF16)
    nc.gpsimd.memset(ident_bf[:], 1.0)
    nc.gpsimd.affine_select(ident_bf[:], ident_bf[:], pattern=[[-1, 128]],
                            compare_op=ALU.is_equal, fill=0.0,
                            base=0, channel_multiplier=1)
    # key-major causal mask, accumulated onto scores by the PE:
    # cols 0:128 / 256:384 are the (s,t)-diagonal blocks (-3200 where t < s,
    # i.e. -400 after the 1/8 softmax scale); cols 128:256 (s0 x t1) unmasked
    mask_bf = const.tile([128, 384], BF16)
    nc.gpsimd.memset(mask_bf[:], 0.0)
    for c0 in (0, 256):
        nc.gpsimd.affine_select(mask_bf[:, c0:c0 + 128], mask_bf[:, c0:c0 + 128],
                                pattern=[[1, 128]], compare_op=ALU.is_ge,
                                fill=-3200.0, base=0, channel_multiplier=-1)

    def pers(pool, name, shape, dtype=F32):
        return pool.tile(shape, dtype, tag=name, name=name)

    # ---------------- weight tiles (DMAs deferred so the prologue overlaps)
    wqk_A = pers(wp, "wqk_A", [128, 4 * C], F8)
    wqk_B = pers(wp, "wqk_B", [128, 4 * C], F8)
    wv_A = pers(wp, "wv_A", [128, 2 * C], F8)
    wv_B = pers(wp, "wv_B", [128, 2 * C], F8)
    wo_t = pers(wp, "wo_t", [128, KC * C], BF16)
    w1_A = pers(wp, "w1_A", [128, 2 * DFF], F8)
    w1_B = pers(wp, "w1_B", [128, 2 * DFF], F8)
    w2_dr = pers(wp, "w2_dr", [128, (KF // 2) * 2 * C], F8)
    _wjobs = [(wqk_A, wqk_A_d), (wqk_B, wqk_B_d), (wv_A, wv_A_d),
              (wv_B, wv_B_d), (wo_t, wo_d), (w1_A, w1_A_d),
              (w1_B, w1_B_d), (w2_dr, w2_dr_d)]
    if flags["b1"]:
        b1T = pers(wp, "b1T", [128, KF])
        _wjobs.append((b1T, b1_d.rearrange("(a p) -> p a", p=128)))

    def load_weights():
        for dst, src in _wjobs:
            nc.sync.dma_start(dst[:], src)

    def bcast_row(nm, src):
        row = pers(wp, f"{nm}_row", [1, C])
        nc.sync.dma_start(row[:], src.rearrange("c -> 1 c"))
        full = pers(wp, f"{nm}_bc", [128, C])
        nc.gpsimd.partition_broadcast(full[:], row[:])
        return full

    bc = {nm: bcast_row(nm, opt[nm]) for nm in opt}

    # weight views for DoubleRow matmuls ([p, i, f] with i the K-pair index)
    wqk_A_v = wqk_A[:].rearrange("p (i f) -> p i f", i=2)
    wqk_B_v = wqk_B[:].rearrange("p (i f) -> p i f", i=2)
    wv_A_v = wv_A[:].rearrange("p (i f) -> p i f", i=2)
    wv_B_v = wv_B[:].rearrange("p (i f) -> p i f", i=2)
    w1_A_v = w1_A[:].rearrange("p (i f) -> p i f", i=2)
    w1_B_v = w1_B[:].rearrange("p (i f) -> p i f", i=2)
    w2_dr_v = w2_dr[:].rearrange("p (m i f) -> p m i f", m=KF // 2, i=2)

    # ---------------- helpers ----------------
    def layernorm_pair(x_ts, g_nm, beta_nm, tag):
        """token-major LN of two [128, C] tiles -> bf16 h tiles.

        rsqrt(var+eps) via DVE Taylor seed + 2 Newton iterations (fp32-exact
        for the ~unit variances here) -- ScalarE keeps the exp/relu/copy
        table set, zero ACT table swaps.
        """
        mvs = []
        var2 = sb.tile([128, NT], F32, tag="var2", bufs=8, name=f"var2_{tag}")
        for tt in range(NT):
            bns = sb.tile([128, 6], F32, tag="bns", bufs=8, name=f"bns_{tag}{tt}")
            nc.vector.bn_stats(bns[:], x_ts[tt][:])
            mv = sb.tile([128, 2], F32, tag="mv", bufs=8, name=f"mv_{tag}{tt}")
            nc.vector.bn_aggr(mv[:], bns[:])
            mvs.append(mv)
            nc.vector.tensor_scalar(var2[:, tt:tt + 1], mv[:, 1:2], EPS, None,
                                    ALU.add)
        y = sb.tile([128, NT], F32, tag="rsy", bufs=8, name=f"rsy_{tag}")
        nc.gpsimd.tensor_scalar(y[:], var2[:], -0.5, 1.5, ALU.mult, ALU.add)
        for it in range(2):
            t1 = sb.tile([128, NT], F32, tag="rst1", bufs=8, name=f"rst1_{tag}{it}")
            nc.gpsimd.tensor_tensor(t1[:], y[:], y[:], op=ALU.mult)
            nc.gpsimd.tensor_tensor(t1[:], t1[:], var2[:], op=ALU.mult)
            nc.gpsimd.tensor_scalar(t1[:], t1[:], -0.5, 1.5, ALU.mult, ALU.add)
            y2 = sb.tile([128, NT], F32, tag="rsy2", bufs=8, name=f"rsy2_{tag}{it}")
            nc.gpsimd.tensor_tensor(y2[:], y[:], t1[:], op=ALU.mult)
            y = y2
        h_ts = []
        for tt in range(NT):
            h_t = sb.tile([128, C], BF16, tag="h", bufs=6, name=f"h_{tag}{tt}")
            nc.gpsimd.tensor_scalar(h_t[:], x_ts[tt][:], mvs[tt][:, 0:1],
                                    y[:, tt:tt + 1], ALU.subtract, ALU.mult)
            if g_nm in bc:
                nc.gpsimd.tensor_tensor(h_t[:], h_t[:], bc[g_nm][:], op=ALU.mult)
            if beta_nm in bc:
                nc.gpsimd.tensor_tensor(h_t[:], h_t[:], bc[beta_nm][:], op=ALU.add)
            h_ts.append(h_t)
        return h_ts

    def hT_passes(hT):
        """[128, 1024] fp8 tile -> the two zero-padded DoubleRow rhs views.

        col = a*256 + t with a in {kt0, kt1, kt2, zeros}; pass A pairs
        (kt0, kt2) i.e. c=(p, 256+p), pass B pairs (kt1, zero-pad)."""
        v4 = hT[:].rearrange("p (i j t) -> p i j t", i=2, j=2)
        pA = v4[:, :, 0:1, :].rearrange("p i j t -> p i (j t)")
        pB = v4[:, :, 1:2, :].rearrange("p i j t -> p i (j t)")
        return pA, pB

    def transpose_to_f8(h_ts, tag, via_pe=None):
        """2 token-major bf16 [128, C] tiles -> one fp8 [128, 1024] tile
        (cols 768:1024 pre-zeroed for the DoubleRow zero-pad pass).

        Default path rides the idle DMA engines + GpSimd conversion; the
        via_pe path (PE transpose + ACT evac) has ~3us less latency and is
        used at the pipeline ramp/drain where those engines are idle."""
        hT = sb.tile([128, 4 * 256], F8, tag=f"hT_{tag[0]}", bufs=5,
                     name=f"hT_{tag}")
        if via_pe:
            tp = ps.tile([128, 2 * C], BF16, tag=via_pe,
                         bufs=2, name=f"tp_{tag}")
            for kt in range(KC):
                for tt in range(NT):
                    nc.tensor.transpose(
                        tp[:, kt * 256 + tt * 128:kt * 256 + tt * 128 + 128],
                        h_ts[tt][:, kt * 128:(kt + 1) * 128], ident_bf[:])
            nc.scalar.copy(hT[:, 0:2 * C], tp[:])
        else:
            tb = sb.tile([128, 2 * C], BF16, tag=f"hTb_{tag[0]}", bufs=4,
                         name=f"hTb_{tag}")
            for kt in range(KC):
                for tt in range(NT):
                    nc.sync.dma_start_transpose(
                        tb[:, kt * 256 + tt * 128:kt * 256 + tt * 128 + 128],
                        h_ts[tt][:, kt * 128:(kt + 1) * 128])
            nc.gpsimd.tensor_copy(hT[:, 0:2 * C], tb[:])
        nc.gpsimd.memset(hT[:, 2 * C:4 * 256], 0.0)
        return hT

    def stage_x(b):
        """load x, LN1, transpose -> fp8 hT for batch b"""
        x_ts = []
        for tt in range(NT):
            x_t = sb.tile([128, C], F32, tag="x", bufs=8, name=f"x_{b}_{tt}")
            nc.sync.dma_start(x_t[:], x_d[b, tt * 128:(tt + 1) * 128, :])
            x_ts.append(x_t)
        h_ts = layernorm_pair(x_ts, "g1", "beta1", f"a{b}")
        hT = transpose_to_f8(h_ts, f"a{b}",
                             via_pe={0: "sc", 1: "mlp"}.get(b))
        return x_ts, hT

    def stage_qkv(b, xh):
        """qkT (bf16, key/query transposed) + v' (bf16, 66-stride groups)"""
        x_ts, hT = xh
        hT_A, hT_B = hT_passes(hT)
        qkT = []
        for mt in range(KC):
            g_ps = ps.tile([128, 512], F32, tag="qv", bufs=2,
                           name=f"qkps_{b}{mt}")
            for half in range(2):  # 0 = q, 1 = k
                sl = slice(half * 256, (half + 1) * 256)
                wsl = slice(half * C + mt * 128, half * C + (mt + 1) * 128)
                nc.tensor.matmul(g_ps[:, sl], wqk_A_v[:, :, wsl],
                                 hT_A, start=True, stop=False, perf_mode=DR)
                nc.tensor.matmul(g_ps[:, sl], wqk_B_v[:, :, wsl],
                                 hT_B, start=False, stop=True, perf_mode=DR)
            o = sb.tile([128, 512], BF16, tag="qkT", bufs=9, name=f"qkT_{b}{mt}")
            if b == 0 and mt != 1:
                nc.scalar.copy(o[:], g_ps[:])   # ramp: ACT is idle
            else:
                nc.vector.tensor_copy(o[:], g_ps[:])
            qkT.append(o)
        v_ts = []
        for tt in range(NT):
            g_ps = ps.tile([128, C], F32, tag="qv", bufs=2, name=f"vps_{b}{tt}")
            nc.tensor.matmul(g_ps[:], hT_A[:, :, tt * 128:(tt + 1) * 128],
                             wv_A_v, start=True, stop=False, perf_mode=DR)
            nc.tensor.matmul(g_ps[:], hT_B[:, :, tt * 128:(tt + 1) * 128],
                             wv_B_v, start=False, stop=True, perf_mode=DR)
            vp = sb.tile([128, H * GA], BF16, tag="v", bufs=8, name=f"v_{b}{tt}")
            vp_g = vp[:].rearrange("p (h a) -> p h a", a=GA)
            nc.scalar.copy(vp_g[:, :, 0:64],
                           g_ps[:].rearrange("p (h a) -> p h a", a=64))
            nc.gpsimd.memset(vp_g[:, :, 64:66], 1.0)
            v_ts.append(vp)
        return dict(x_ts=x_ts, qkT=qkT, v_ts=v_ts)

    def attn_scores(b, st):
        """key-major scores -> PE mask -> exp : bf16 expT views per head.

        Two heads share one 2-bank PSUM tile (head A in bank 0, head B in
        bank 1) so each exp instruction covers both via a strided AP."""
        qkT = st["qkT"]
        ees = []
        for h in range(H):
            pr, off = h // 2, 64 * (h % 2)
            qk = qkT[pr]
            s_ps = ps.tile([128, 384], F32, tag="sc", bufs=2, name=f"sps_{b}{h}")
            # S^T[s, t] = kT.T @ qT ; (s1, t0) block skipped (fully masked)
            nc.tensor.matmul(s_ps[:, 0:256], qk[off:off + 64, 256:384],
                             qk[off:off + 64, 0:256], start=True, stop=False)
            nc.tensor.matmul(s_ps[:, 256:384], qk[off:off + 64, 384:512],
                             qk[off:off + 64, 128:256], start=False, stop=False)
            # causal mask accumulated by the PE (no cross-engine hop)
            nc.tensor.matmul(s_ps[:], ident_bf[:], mask_bf[:],
                             start=False, stop=True)
            ee = sb.tile([128, 384], BF16, tag="ee", bufs=12, name=f"ee_{b}{h}")
            nc.scalar.activation(ee[:], s_ps[:], AF.Exp, scale=0.125)
            ees.append((ee, 0))
        return ees

    def attn_av(b, st, ees):
        """AV (+denominator col) -> strided normalize -> DMA transpose
        -> attnT [128, 768] bf16"""
        v_ts = st["v_ts"]
        at_ps = [ps.tile([128, H * GA], F32, tag="aw", bufs=2,
                         name=f"atps_{b}{tt}") for tt in range(NT)]
        attnT = sb.tile([128, 2 * C], BF16, tag="attnT", bufs=4,
                        name=f"attnT_{b}")
        for tt in range(NT):  # t0's AV group completes (and frees) first
            for h in range(H):
                ee, e0 = ees[h]
                cols = slice(h * GA, (h + 1) * GA)
                if tt == 0:
                    nc.tensor.matmul(at_ps[0][:, cols], ee[:, e0:e0 + 128],
                                     v_ts[0][:, cols], start=True, stop=True)
                else:
                    nc.tensor.matmul(at_ps[1][:, cols], ee[:, e0 + 128:e0 + 256],
                                     v_ts[0][:, cols], start=True, stop=False)
                    nc.tensor.matmul(at_ps[1][:, cols], ee[:, e0 + 256:e0 + 384],
                                     v_ts[1][:, cols], start=False, stop=True)
            at_g = at_ps[tt][:].rearrange("p (h a) -> p h a", a=GA)
            r = sb.tile([128, H], F32, tag="r", bufs=6, name=f"r_{b}{tt}")
            nc.vector.reciprocal(r[:], at_g[:, :, 64:65]
                                 .rearrange("p h a -> p (h a)"))
            a_sb = sb.tile([128, C], BF16, tag="asb", bufs=6, name=f"asb_{b}{tt}")
            nc.vector.tensor_tensor(
                a_sb[:].rearrange("p (h a) -> p h a", a=64),
                at_g[:, :, 0:64],
                r[:].rearrange("p (h o) -> p h o", o=1).broadcast_to([128, H, 64]),
                op=ALU.mult)
            for kt in range(KC):
                nc.sync.dma_start_transpose(
                    attnT[:, kt * 256 + tt * 128:kt * 256 + tt * 128 + 128],
                    a_sb[:, kt * 128:(kt + 1) * 128])
        return attnT

    def wo_ln2(b, st, attnT):
        """Wo projection + residual + LN2 -> (x2 tiles, fp8 h2T)"""
        x_ts = st["x_ts"]
        x2_ts = []
        for tt in range(NT):
            g_ps = ps.tile([128, C], F32, tag="aw", bufs=2, name=f"pps_{b}{tt}")
            for kt in range(KC):
                nc.tensor.matmul(g_ps[:],
                                 attnT[:, kt * 256 + tt * 128:kt * 256 + tt * 128 + 128],
                                 wo_t[:, kt * C:(kt + 1) * C],
                                 start=(kt == 0), stop=(kt == KC - 1))
            x2 = sb.tile([128, C], F32, tag="x2", bufs=10, name=f"x2_{b}{tt}")
            nc.vector.tensor_tensor(x2[:], g_ps[:], x_ts[tt][:], op=ALU.add)
            if "bo" in bc:
                nc.vector.tensor_tensor(x2[:], x2[:], bc["bo"][:], op=ALU.add)
            x2_ts.append(x2)

        h2_ts = layernorm_pair(x2_ts, "g2", "beta2", f"m{b}")
        h2T = transpose_to_f8(h2_ts, f"m{b}",
                              via_pe="sc" if b == BL - 1 else None)
        return x2_ts, h2T

    def tail_mlp(b, x2_ts, h2T):
        """W1 -> relu -> W2 -> residual out (runs one pipeline step behind)"""
        h2T_A, h2T_B = hT_passes(h2T)
        ffT = []
        for mp in range(KF // 2):  # pairs of m-tiles share one PSUM bank
            f_ps = ps.tile([128, 512], F32, tag="mlp", bufs=2, name=f"fps_{b}{mp}")
            for half in range(2):
                mt = mp * 2 + half
                sl = slice(half * 256, (half + 1) * 256)
                nc.tensor.matmul(f_ps[:, sl],
                                 w1_A_v[:, :, mt * 128:(mt + 1) * 128],
                                 h2T_A, start=True, stop=False, perf_mode=DR)
                nc.tensor.matmul(f_ps[:, sl],
                                 w1_B_v[:, :, mt * 128:(mt + 1) * 128],
                                 h2T_B, start=False, stop=True, perf_mode=DR)
            o = sb.tile([128, 512], F8, tag="ffT", bufs=8, name=f"ffT_{b}{mp}")
            if flags["b1"]:
                for half in range(2):
                    mt = mp * 2 + half
                    sl = slice(half * 256, (half + 1) * 256)
                    nc.scalar.activation(o[:, sl], f_ps[:, sl], AF.Relu,
                                         bias=b1T[:, mt:mt + 1])
            elif b == BL - 1 and mp % 2 == 1:
                # drain: split last batch's relus across ACT and DVE
                nc.vector.tensor_scalar(o[:], f_ps[:], 0.0, None, ALU.max)
            else:
                nc.scalar.activation(o[:], f_ps[:], AF.Relu)
            ffT.append(o)
        for tt in range(NT):
            g_ps = ps.tile([128, C], F32, tag="mlp", bufs=2, name=f"f2ps_{b}{tt}")
            for mp in range(KF // 2):
                lhsT = (ffT[mp][:].rearrange("p (i t) -> p i t", i=2)
                        [:, :, tt * 128:(tt + 1) * 128])
                nc.tensor.matmul(g_ps[:], lhsT, w2_dr_v[:, mp],
                                 start=(mp == 0), stop=(mp == KF // 2 - 1),
                                 perf_mode=DR)
            o = sb.tile([128, C], F32, tag="outt", bufs=6, name=f"o_{b}{tt}")
            nc.vector.tensor_tensor(o[:], g_ps[:], x2_ts[tt][:], op=ALU.add)
            if "b2" in bc:
                nc.vector.tensor_tensor(o[:], o[:], bc["b2"][:], op=ALU.add)
            nc.sync.dma_start(out_d[b, tt * 128:(tt + 1) * 128, :], o[:])

    # ---------------- main loop (phase-pipelined: batch b's MLP is deferred
    # into iteration b+1 so its relus queue behind batch b+1's exps on ACT) --
    for _rep in range(repeat):
        xh = {0: stage_x(0)}
        if _rep == 0:
            load_weights()
        xh[1] = stage_x(1)
        st = {0: stage_qkv(0, xh[0])}
        xh[2] = stage_x(2)
        for b in range(BL):
            cur = st.pop(b)
            ees = attn_scores(b, cur)
            attnT = attn_av(b, cur, ees)
            if b + 1 < BL:
                st[b + 1] = stage_qkv(b + 1, xh.pop(b + 1))
            if b + 3 < BL:
                xh[b + 3] = stage_x(b + 3)
            x2_ts, h2T = wo_ln2(b, cur, attnT)
            tail_mlp(b, x2_ts, h2T)


_CACHED = {}


def build(flags_key, flags, repeat=1):
    key = (flags_key, repeat)
    if key in _CACHED:
        return _CACHED[key]
    nc = bacc.Bacc("TRN2", target_bir_lowering=False, debug=False,
                   enable_asserts=False, num_devices=N_CORES)
    with tile.TileContext(nc) as tc:
        block_kernel(tc, flags, repeat=repeat)
    nc.compile()
    _CACHED[key] = nc
    return nc


def _flags(inputs):
    return {
        "b1": not np.allclose(inputs["b1"], 0.0),
        "bo": not np.allclose(inputs["bo"], 0.0),
        "b2": not np.allclose(inputs["b2"], 0.0),
        "g1": not np.allclose(inputs["g1"], 1.0),
        "beta1": not np.allclose(inputs["beta1"], 0.0),
        "g2": not np.allclose(inputs["g2"], 1.0),
        "beta2": not np.allclose(inputs["beta2"], 0.0),
    }


def _prep_weights(inputs):
    """host-side weight layouts (fp8/bf16) keyed by dram tensor name"""
    Wq = np.transpose(inputs["Wq"], (1, 0, 2)).reshape(C, C)   # [c, (h hs)]
    Wk = np.transpose(inputs["Wk"], (1, 0, 2)).reshape(C, C)
    Wv = np.transpose(inputs["Wv"], (1, 0, 2)).reshape(C, C)
    Wo, W1, W2 = inputs["Wo"], inputs["W1"], inputs["W2"]
    f8 = lambda a: np.ascontiguousarray(a).astype(NPF8)
    z = np.zeros((128, C), np.float32)
    zf = np.zeros((128, DFF), np.float32)
    # pass A pairs K-rows (p, 256+p); pass B pairs (128+p, zero-pad)
    out = {
        "wqk_A": f8(np.concatenate(
            [Wq[0:128], Wk[0:128], Wq[256:384], Wk[256:384]], axis=1)),
        "wqk_B": f8(np.concatenate(
            [Wq[128:256], Wk[128:256], z, z], axis=1)),
        "wv_A": f8(np.concatenate([Wv[0:128], Wv[256:384]], axis=1)),
        "wv_B": f8(np.concatenate([Wv[128:256], z], axis=1)),
        "wo_t": np.ascontiguousarray(np.concatenate(
            [Wo[kt * 128:(kt + 1) * 128] for kt in range(KC)],
            axis=1)).astype(NPBF16),
        "w1_A": f8(np.concatenate([W1[0:128], W1[256:384]], axis=1)),
        "w1_B": f8(np.concatenate([W1[128:256], zf], axis=1)),
        "w2_dr": f8(np.concatenate(
            [np.concatenate([W2[mp * 256:mp * 256 + 128],
                             W2[mp * 256 + 128:mp * 256 + 256]], axis=1)
             for mp in range(KF // 2)], axis=1)),
    }
    return out


def make_in_maps(inputs, needed):
    prep = _prep_weights(inputs)
    in_maps = []
    for c in range(N_CORES):
        m = {}
        for nm in needed:
            if nm == "x":
                m[nm] = inputs["x"][c * BL:(c + 1) * BL]
            elif nm in prep:
                m[nm] = prep[nm]
            else:
                m[nm] = inputs[nm]
        in_maps.append(m)
    return in_maps


def needed_inputs(nc):
    needed = set()
    for alloc in nc.m.functions[0].allocations:
        if isinstance(alloc, mybir.MemoryLocationSet) and alloc.kind == "ExternalInput":
            nm = alloc.memorylocations[0].name
            if nm != "partition_id":
                needed.add(nm)
    return needed


def kernel(**inputs):
    inputs = {k: np.ascontiguousarray(np.asarray(v, dtype=np.float32))
              for k, v in inputs.items()}
    flags = _flags(inputs)
    key = tuple(sorted(flags.items()))
    nc = build(key, flags)
    in_maps = make_in_maps(inputs, needed_inputs(nc))
    res = run_bass_kernel_spmd(nc, in_maps, core_ids=list(range(N_CORES)))
    out = np.concatenate([res.results[c]["out"] for c in range(N_CORES)], axis=0)
    return out


# revision 52
# speedup vs baseline: 1.0707x; 1.0707x over previous
"""Trainium2 Bass kernel for a dense transformer block (pre-LN, causal MHA + MLP).

Problem: x[64,256,384], 6 heads x 64, d_ff=1536.
Strategy: pure data parallel over batch -- each of 8 NeuronCores processes 8
batches with replicated weights; no collectives.

Design (vs the fp32r baseline, ~2.5x faster in CoreSim):
  * fp8(e4m3) weights pre-cast on the host; every QKV / W1 / W2 GEMM pass
    runs fp8 DoubleRow at 0.5 cyc/row -- K=384 contractions are padded to
    512 with zero weight rows so both passes pair 2 K-rows per PE cell.
  * key-major attention: scores computed directly as S^T[s,t] = kT.T @ qT
    (causal (s1,t0) block never computed; triangular mask accumulated onto
    the scores by a PE matmul so no cross-engine hop before exp). exp()
    output is already the AV matmul's lhsT layout -- no wei transposes.
  * softmax denominators ride along as a ones-column appended to V (column
    64 of each 66-wide head group); normalization happens token-major in
    one strided DVE multiply per token tile, after the AV matmul.
  * h / attn transposes ride the idle DMA engines (dma_start_transpose,
    bf16) with fp8 conversion on idle GpSimd; LayerNorm scale/offset and
    the Newton rsqrt also live on GpSimd.  PE / ACT(exp,relu) / DVE(LN
    stats, PSUM evacuations) end up near-balanced at ~63 us busy each.
  * PSUM is split into four 2-bank pools, one per GEMM phase (qk+v /
    scores / AV+Wo / MLP) -- a shared pool's buffer rotation otherwise
    serializes independent phases and halves throughput.
  * pipeline ramp/drain use latency-optimized variants (PE-transposes,
    ACT/DVE-split relus and qkT evacuations) since those engines idle there.
"""
import numpy as np
import ml_dtypes
from contextlib import ExitStack

import concourse.bass as bass
import concourse.tile as tile
from concourse import bacc, mybir
from concourse._compat import with_exitstack
from concourse.bass_utils import run_bass_kernel_spmd

F32 = mybir.dt.float32
BF16 = mybir.dt.bfloat16
F8 = mybir.dt.float8e4
AF = mybir.ActivationFunctionType
ALU = mybir.AluOpType
DR = mybir.MatmulPerfMode.DoubleRow

NPF8 = ml_dtypes.float8_e4m3
NPBF16 = ml_dtypes.bfloat16

N_CORES = 8
B, T, C = 64, 256, 384
H, HS = 6, 64
DFF = 4 * C
EPS = 1e-5
BL = B // N_CORES          # 8 batches per core
NT = T // 128              # 2 token-tiles per batch
KC = C // 128              # 3 feature tiles
KF = DFF // 128            # 12 ff tiles
GA = 66                    # attn head group stride (64 attn + d + pad)


@with_exitstack
def block_kernel(ctx: ExitStack, tc: tile.TileContext, flags: dict, repeat: int = 1):
    nc = tc.nc
    x_d = nc.dram_tensor("x", [BL, T, C], F32, kind="ExternalInput").ap()
    wqk_A_d = nc.dram_tensor("wqk_A", [128, 4 * C], F8, kind="ExternalInput").ap()
    wqk_B_d = nc.dram_tensor("wqk_B", [128, 4 * C], F8, kind="ExternalInput").ap()
    wv_A_d = nc.dram_tensor("wv_A", [128, 2 * C], F8, kind="ExternalInput").ap()
    wv_B_d = nc.dram_tensor("wv_B", [128, 2 * C], F8, kind="ExternalInput").ap()
    wo_d = nc.dram_tensor("wo_t", [128, KC * C], BF16, kind="ExternalInput").ap()
    w1_A_d = nc.dram_tensor("w1_A", [128, 2 * DFF], F8, kind="ExternalInput").ap()
    w1_B_d = nc.dram_tensor("w1_B", [128, 2 * DFF], F8, kind="ExternalInput").ap()
    w2_dr_d = nc.dram_tensor("w2_dr", [128, (KF // 2) * 2 * C], F8,
                             kind="ExternalInput").ap()
    b1_d = (nc.dram_tensor("b1", [DFF], F32, kind="ExternalInput").ap()
            if flags["b1"] else None)
    out_d = nc.dram_tensor("out", [BL, T, C], F32, kind="ExternalOutput").ap()
    opt = {}
    for nm in ["bo", "b2", "g1", "beta1", "g2", "beta2"]:
        if flags[nm]:
            opt[nm] = nc.dram_tensor(nm, [C], F32, kind="ExternalInput").ap()

    const = ctx.enter_context(tc.tile_pool(name="const", bufs=1))
    wp = ctx.enter_context(tc.tile_pool(name="wp", bufs=1))
    sb = ctx.enter_context(tc.tile_pool(name="sb", bufs=1))
    ps = ctx.enter_context(tc.tile_pool(name="ps", bufs=1, space="PSUM"))

    # ---------------- constants ----------------
    ident_bf = const.tile([128, 128], BF16)
    nc.gpsimd.memset(ident_bf[:], 1.0)
    nc.gpsimd.affine_select(ident_bf[:], ident_bf[:], pattern=[[-1, 128]],
                            compare_op=ALU.is_equal, fill=0.0,
                            base=0, channel_multiplier=1)
    # key-major causal mask, accumulated onto scores by the PE:
    # cols 0:128 / 256:384 are the (s,t)-diagonal blocks (-3200 where t < s,
    # i.e. -400 after the 1/8 softmax scale); cols 128:256 (s0 x t1) unmasked
    mask_bf = const.tile([128, 384], BF16)
    nc.gpsimd.memset(mask_bf[:], 0.0)
    for c0 in (0, 256):
        nc.gpsimd.affine_select(mask_bf[:, c0:c0 + 128], mask_bf[:, c0:c0 + 128],
                                pattern=[[1, 128]], compare_op=ALU.is_ge,
                                fill=-3200.0, base=0, channel_multiplier=-1)

    def pers(pool, name, shape, dtype=F32):
        return pool.tile(shape, dtype, tag=name, name=name)

    # ---------------- weight tiles (DMAs deferred so the prologue overlaps)
    wqk_A = pers(wp, "wqk_A", [128, 4 * C], F8)
    wqk_B = pers(wp, "wqk_B", [128, 4 * C], F8)
    wv_A = pers(wp, "wv_A", [128, 2 * C], F8)
    wv_B = pers(wp, "wv_B", [128, 2 * C], F8)
    wo_t = pers(wp, "wo_t", [128, KC * C], BF16)
    w1_A = pers(wp, "w1_A", [128, 2 * DFF], F8)
    w1_B = pers(wp, "w1_B", [128, 2 * DFF], F8)
    w2_dr = pers(wp, "w2_dr", [128, (KF // 2) * 2 * C], F8)
    _wjobs = [(wqk_A, wqk_A_d), (wqk_B, wqk_B_d), (wv_A, wv_A_d),
              (wv_B, wv_B_d), (wo_t, wo_d), (w1_A, w1_A_d),
              (w1_B, w1_B_d), (w2_dr, w2_dr_d)]
    if flags["b1"]:
        b1T = pers(wp, "b1T", [128, KF])
        _wjobs.append((b1T, b1_d.rearrange("(a p) -> p a", p=128)))

    def load_weights():
        for dst, src in _wjobs:
            nc.sync.dma_start(dst[:], src)

    def bcast_row(nm, src):
        row = pers(wp, f"{nm}_row", [1, C])
        nc.sync.dma_start(row[:], src.rearrange("c -> 1 c"))
        full = pers(wp, f"{nm}_bc", [128, C])
        nc.gpsimd.partition_broadcast(full[:], row[:])
        return full

    bc = {nm: bcast_row(nm, opt[nm]) for nm in opt}

    # weight views for DoubleRow matmuls ([p, i, f] with i the K-pair index)
    wqk_A_v = wqk_A[:].rearrange("p (i f) -> p i f", i=2)
    wqk_B_v = wqk_B[:].rearrange("p (i f) -> p i f", i=2)
    wv_A_v = wv_A[:].rearrange("p (i f) -> p i f", i=2)
    wv_B_v = wv_B[:].rearrange("p (i f) -> p i f", i=2)
    w1_A_v = w1_A[:].rearrange("p (i f) -> p i f", i=2)
    w1_B_v = w1_B[:].rearrange("p (i f) -> p i f", i=2)
    w2_dr_v = w2_dr[:].rearrange("p (m i f) -> p m i f", m=KF // 2, i=2)

    # ---------------- helpers ----------------
    def layernorm_pair(x_ts, g_nm, beta_nm, tag):
        """token-major LN of two [128, C] tiles -> bf16 h tiles.

        rsqrt(var+eps) via DVE Taylor seed + 2 Newton iterations (fp32-exact
        for the ~unit variances here) -- ScalarE keeps the exp/relu/copy
        table set, zero ACT table swaps.
        """
        mvs = []
        var2 = sb.tile([128, NT], F32, tag="var2", bufs=8, name=f"var2_{tag}")
        for tt in range(NT):
            bns = sb.tile([128, 6], F32, tag="bns", bufs=8, name=f"bns_{tag}{tt}")
            nc.vector.bn_stats(bns[:], x_ts[tt][:])
            mv = sb.tile([128, 2], F32, tag="mv", bufs=8, name=f"mv_{tag}{tt}")
            nc.vector.bn_aggr(mv[:], bns[:])
            mvs.append(mv)
            nc.vector.tensor_scalar(var2[:, tt:tt + 1], mv[:, 1:2], EPS, None,
                                    ALU.add)
        y = sb.tile([128, NT], F32, tag="rsy", bufs=8, name=f"rsy_{tag}")
        nc.gpsimd.tensor_scalar(y[:], var2[:], -0.5, 1.5, ALU.mult, ALU.add)
        for it in range(2):
            t1 = sb.tile([128, NT], F32, tag="rst1", bufs=8, name=f"rst1_{tag}{it}")
            nc.gpsimd.tensor_tensor(t1[:], y[:], y[:], op=ALU.mult)
            nc.gpsimd.tensor_tensor(t1[:], t1[:], var2[:], op=ALU.mult)
            nc.gpsimd.tensor_scalar(t1[:], t1[:], -0.5, 1.5, ALU.mult, ALU.add)
            y2 = sb.tile([128, NT], F32, tag="rsy2", bufs=8, name=f"rsy2_{tag}{it}")
            nc.gpsimd.tensor_tensor(y2[:], y[:], t1[:], op=ALU.mult)
            y = y2
        h_ts = []
        for tt in range(NT):
            h_t = sb.tile([128, C], BF16, tag="h", bufs=6, name=f"h_{tag}{tt}")
            nc.gpsimd.tensor_scalar(h_t[:], x_ts[tt][:], mvs[tt][:, 0:1],
                                    y[:, tt:tt + 1], ALU.subtract, ALU.mult)
            if g_nm in bc:
                nc.gpsimd.tensor_tensor(h_t[:], h_t[:], bc[g_nm][:], op=ALU.mult)
            if beta_nm in bc:
                nc.gpsimd.tensor_tensor(h_t[:], h_t[:], bc[beta_nm][:], op=ALU.add)
            h_ts.append(h_t)
        return h_ts

    def hT_passes(hT):
        """[128, 1024] fp8 tile -> the two zero-padded DoubleRow rhs views.

        col = a*256 + t with a in {kt0, kt1, kt2, zeros}; pass A pairs
        (kt0, kt2) i.e. c=(p, 256+p), pass B pairs (kt1, zero-pad)."""
        v4 = hT[:].rearrange("p (i j t) -> p i j t", i=2, j=2)
        pA = v4[:, :, 0:1, :].rearrange("p i j t -> p i (j t)")
        pB = v4[:, :, 1:2, :].rearrange("p i j t -> p i (j t)")
        return pA, pB

    def transpose_to_f8(h_ts, tag, via_pe=None):
        """2 token-major bf16 [128, C] tiles -> one fp8 [128, 1024] tile
        (cols 768:1024 pre-zeroed for the DoubleRow zero-pad pass).

        Default path rides the idle DMA engines + GpSimd conversion; the
        via_pe path (PE transpose + ACT evac) has ~3us less latency and is
        used at the pipeline ramp/drain where those engines are idle."""
        hT = sb.tile([128, 4 * 256], F8, tag=f"hT_{tag[0]}", bufs=5,
                     name=f"hT_{tag}")
        if via_pe:
            tp = ps.tile([128, 2 * C], BF16, tag=via_pe,
                         bufs=2, name=f"tp_{tag}")
            for kt in range(KC):
                for tt in range(NT):
                    nc.tensor.transpose(
                        tp[:, kt * 256 + tt * 128:kt * 256 + tt * 128 + 128],
                        h_ts[tt][:, kt * 128:(kt + 1) * 128], ident_bf[:])
            nc.scalar.copy(hT[:, 0:2 * C], tp[:])
        else:
            tb = sb.tile([128, 2 * C], BF16, tag=f"hTb_{tag[0]}", bufs=4,
                         name=f"hTb_{tag}")
            for kt in range(KC):
                for tt in range(NT):
                    nc.sync.dma_start_transpose(
                        tb[:, kt * 256 + tt * 128:kt * 256 + tt * 128 + 128],
                        h_ts[tt][:, kt * 128:(kt + 1) * 128])
            nc.gpsimd.tensor_copy(hT[:, 0:2 * C], tb[:])
        nc.gpsimd.memset(hT[:, 2 * C:4 * 256], 0.0)
        return hT

    def stage_x(b):
        """load x, LN1, transpose -> fp8 hT for batch b"""
        x_ts = []
        for tt in range(NT):
            x_t = sb.tile([128, C], F32, tag="x", bufs=8, name=f"x_{b}_{tt}")
            nc.sync.dma_start(x_t[:], x_d[b, tt * 128:(tt + 1) * 128, :])
            x_ts.append(x_t)
        h_ts = layernorm_pair(x_ts, "g1", "beta1", f"a{b}")
        hT = transpose_to_f8(h_ts, f"a{b}",
                             via_pe={0: "sc", 1: "mlp"}.get(b))
        return x_ts, hT

    def stage_qkv(b, xh):
        """qkT (bf16, key/query transposed) + v' (bf16, 66-stride groups)"""
        x_ts, hT = xh
        hT_A, hT_B = hT_passes(hT)
        qkT = []
        for mt in range(KC):
            g_ps = ps.tile([128, 512], F32, tag="qv", bufs=2,
                           name=f"qkps_{b}{mt}")
            for half in range(2):  # 0 = q, 1 = k
                sl = slice(half * 256, (half + 1) * 256)
                wsl = slice(half * C + mt * 128, half * C + (mt + 1) * 128)
                nc.tensor.matmul(g_ps[:, sl], wqk_A_v[:, :, wsl],
                                 hT_A, start=True, stop=False, perf_mode=DR)
                nc.tensor.matmul(g_ps[:, sl], wqk_B_v[:, :, wsl],
                                 hT_B, start=False, stop=True, perf_mode=DR)
            o = sb.tile([128, 512], BF16, tag="qkT", bufs=9, name=f"qkT_{b}{mt}")
            if b == 0 and mt != 1:
                nc.scalar.copy(o[:], g_ps[:])   # ramp: ACT is idle
            else:
                nc.vector.tensor_copy(o[:], g_ps[:])
            qkT.append(o)
        v_ts = []
        for tt in range(NT):
            g_ps = ps.tile([128, C], F32, tag="qv", bufs=2, name=f"vps_{b}{tt}")
            nc.tensor.matmul(g_ps[:], hT_A[:, :, tt * 128:(tt + 1) * 128],
                             wv_A_v, start=True, stop=False, perf_mode=DR)
            nc.tensor.matmul(g_ps[:], hT_B[:, :, tt * 128:(tt + 1) * 128],
                             wv_B_v, start=False, stop=True, perf_mode=DR)
            vp = sb.tile([128, H * GA], BF16, tag="v", bufs=8, name=f"v_{b}{tt}")
            vp_g = vp[:].rearrange("p (h a) -> p h a", a=GA)
            nc.scalar.copy(vp_g[:, :, 0:64],
                           g_ps[:].rearrange("p (h a) -> p h a", a=64))
            nc.gpsimd.memset(vp_g[:, :, 64:66], 1.0)
            v_ts.append(vp)
        return dict(x_ts=x_ts, qkT=qkT, v_ts=v_ts)

    def attn_scores(b, st):
        """key-major scores -> PE mask -> exp : bf16 expT views per head.

        Two heads share one 2-bank PSUM tile (head A in bank 0, head B in
        bank 1) so each exp instruction covers both via a strided AP."""
        qkT = st["qkT"]
        ees = []
        for h in range(H):
            pr, off = h // 2, 64 * (h % 2)
            qk = qkT[pr]
            s_ps = ps.tile([128, 384], F32, tag="sc", bufs=2, name=f"sps_{b}{h}")
            # S^T[s, t] = kT.T @ qT ; (s1, t0) block skipped (fully masked)
            nc.tensor.matmul(s_ps[:, 0:256], qk[off:off + 64, 256:384],
                             qk[off:off + 64, 0:256], start=True, stop=False)
            nc.tensor.matmul(s_ps[:, 256:384], qk[off:off + 64, 384:512],
                             qk[off:off + 64, 128:256], start=False, stop=False)
            # causal mask accumulated by the PE (no cross-engine hop)
            nc.tensor.matmul(s_ps[:], ident_bf[:], mask_bf[:],
                             start=False, stop=True)
            ee = sb.tile([128, 384], BF16, tag="ee", bufs=12, name=f"ee_{b}{h}")
            nc.scalar.activation(ee[:], s_ps[:], AF.Exp, scale=0.125)
            ees.append((ee, 0))
        return ees

    def attn_av(b, st, ees):
        """AV (+denominator col) -> strided normalize -> DMA transpose
        -> attnT [128, 768] bf16"""
        v_ts = st["v_ts"]
        at_ps = [ps.tile([128, H * GA], F32, tag="aw", bufs=2,
                         name=f"atps_{b}{tt}") for tt in range(NT)]
        attnT = sb.tile([128, 2 * C], BF16, tag="attnT", bufs=4,
                        name=f"attnT_{b}")
        for tt in range(NT):  # t0's AV group completes (and frees) first
            for h in range(H):
                ee, e0 = ees[h]
                cols = slice(h * GA, (h + 1) * GA)
                if tt == 0:
                    nc.tensor.matmul(at_ps[0][:, cols], ee[:, e0:e0 + 128],
                                     v_ts[0][:, cols], start=True, stop=True)
                else:
                    nc.tensor.matmul(at_ps[1][:, cols], ee[:, e0 + 128:e0 + 256],
                                     v_ts[0][:, cols], start=True, stop=False)
                    nc.tensor.matmul(at_ps[1][:, cols], ee[:, e0 + 256:e0 + 384],
                                     v_ts[1][:, cols], start=False, stop=True)
            at_g = at_ps[tt][:].rearrange("p (h a) -> p h a", a=GA)
            r = sb.tile([128, H], F32, tag="r", bufs=6, name=f"r_{b}{tt}")
            nc.vector.reciprocal(r[:], at_g[:, :, 64:65]
                                 .rearrange("p h a -> p (h a)"))
            a_sb = sb.tile([128, C], BF16, tag="asb", bufs=6, name=f"asb_{b}{tt}")
            nc.vector.tensor_tensor(
                a_sb[:].rearrange("p (h a) -> p h a", a=64),
                at_g[:, :, 0:64],
                r[:].rearrange("p (h o) -> p h o", o=1).broadcast_to([128, H, 64]),
                op=ALU.mult)
            for kt in range(KC):
                nc.sync.dma_start_transpose(
                    attnT[:, kt * 256 + tt * 128:kt * 256 + tt * 128 + 128],
                    a_sb[:, kt * 128:(kt + 1) * 128])
        return attnT

    def wo_ln2(b, st, attnT):
        """Wo projection + residual + LN2 -> (x2 tiles, fp8 h2T)"""
        x_ts = st["x_ts"]
        x2_ts = []
        for tt in range(NT):
            g_ps = ps.tile([128, C], F32, tag="aw", bufs=2, name=f"pps_{b}{tt}")
            for kt in range(KC):
                nc.tensor.matmul(g_ps[:],
                                 attnT[:, kt * 256 + tt * 128:kt * 256 + tt * 128 + 128],
                                 wo_t[:, kt * C:(kt + 1) * C],
                                 start=(kt == 0), stop=(kt == KC - 1))
            x2 = sb.tile([128, C], F32, tag="x2", bufs=10, name=f"x2_{b}{tt}")
            nc.vector.tensor_tensor(x2[:], g_ps[:], x_ts[tt][:], op=ALU.add)
            if "bo" in bc:
                nc.vector.tensor_tensor(x2[:], x2[:], bc["bo"][:], op=ALU.add)
            x2_ts.append(x2)

        h2_ts = layernorm_pair(x2_ts, "g2", "beta2", f"m{b}")
        h2T = transpose_to_f8(h2_ts, f"m{b}",
                              via_pe="sc" if b == BL - 1 else None)
        return x2_ts, h2T

    def tail_mlp(b, x2_ts, h2T):
        """W1 -> relu -> W2 -> residual out (runs one pipeline step behind)"""
        h2T_A, h2T_B = hT_passes(h2T)
        ffT = []
        for mp in range(KF // 2):  # pairs of m-tiles share one PSUM bank
            f_ps = ps.tile([128, 512], F32, tag="mlp", bufs=2, name=f"fps_{b}{mp}")
            for half in range(2):
                mt = mp * 2 + half
                sl = slice(half * 256, (half + 1) * 256)
                nc.tensor.matmul(f_ps[:, sl],
                                 w1_A_v[:, :, mt * 128:(mt + 1) * 128],
                                 h2T_A, start=True, stop=False, perf_mode=DR)
                nc.tensor.matmul(f_ps[:, sl],
                                 w1_B_v[:, :, mt * 128:(mt + 1) * 128],
                                 h2T_B, start=False, stop=True, perf_mode=DR)
            o = sb.tile([128, 512], F8, tag="ffT", bufs=8, name=f"ffT_{b}{mp}")
            if flags["b1"]:
                for half in range(2):
                    mt = mp * 2 + half
                    sl = slice(half * 256, (half + 1) * 256)
                    nc.scalar.activation(o[:, sl], f_ps[:, sl], AF.Relu,
                                         bias=b1T[:, mt:mt + 1])
            elif b >= BL - 2 and mp % 2 == 1:
                # drain: split last batch's relus across ACT and DVE
                nc.vector.tensor_scalar(o[:], f_ps[:], 0.0, None, ALU.max)
            else:
                nc.scalar.activation(o[:], f_ps[:], AF.Relu)
            ffT.append(o)
        for tt in range(NT):
            g_ps = ps.tile([128, C], F32, tag="mlp", bufs=2, name=f"f2ps_{b}{tt}")
            for mp in range(KF // 2):
                lhsT = (ffT[mp][:].rearrange("p (i t) -> p i t", i=2)
                        [:, :, tt * 128:(tt + 1) * 128])
                nc.tensor.matmul(g_ps[:], lhsT, w2_dr_v[:, mp],
                                 start=(mp == 0), stop=(mp == KF // 2 - 1),
                                 perf_mode=DR)
            o = sb.tile([128, C], F32, tag="outt", bufs=6, name=f"o_{b}{tt}")
            nc.vector.tensor_tensor(o[:], g_ps[:], x2_ts[tt][:], op=ALU.add)
            if "b2" in bc:
                nc.vector.tensor_tensor(o[:], o[:], bc["b2"][:], op=ALU.add)
            nc.sync.dma_start(out_d[b, tt * 128:(tt + 1) * 128, :], o[:])

    # ---------------- main loop (phase-pipelined: batch b's MLP is deferred
    # into iteration b+1 so its relus queue behind batch b+1's exps on ACT) --
    for _rep in range(repeat):
        xh = {0: stage_x(0)}
        if _rep == 0:
            load_weights()
        xh[1] = stage_x(1)
        st = {0: stage_qkv(0, xh[0])}
        xh[2] = stage_x(2)
        for b in range(BL):
            cur = st.pop(b)
            ees = attn_scores(b, cur)
            attnT = attn_av(b, cur, ees)
            if b + 1 < BL:
                st[b + 1] = stage_qkv(b + 1, xh.pop(b + 1))
            if b + 3 < BL:
                xh[b + 3] = stage_x(b + 3)
            x2_ts, h2T = wo_ln2(b, cur, attnT)
            tail_mlp(b, x2_ts, h2T)


_CACHED = {}


def build(flags_key, flags, repeat=1):
    key = (flags_key, repeat)
    if key in _CACHED:
        return _CACHED[key]
    nc = bacc.Bacc("TRN2", target_bir_lowering=False, debug=False,
                   enable_asserts=False, num_devices=N_CORES)
    with tile.TileContext(nc) as tc:
        block_kernel(tc, flags, repeat=repeat)
    nc.compile()
    _CACHED[key] = nc
    return nc


def _flags(inputs):
    return {
        "b1": not np.allclose(inputs["b1"], 0.0),
        "bo": not np.allclose(inputs["bo"], 0.0),
        "b2": not np.allclose(inputs["b2"], 0.0),
        "g1": not np.allclose(inputs["g1"], 1.0),
        "beta1": not np.allclose(inputs["beta1"], 0.0),
        "g2": not np.allclose(inputs["g2"], 1.0),
        "beta2": not np.allclose(inputs["beta2"], 0.0),
    }


def _prep_weights(inputs):
    """host-side weight layouts (fp8/bf16) keyed by dram tensor name"""
    Wq = np.transpose(inputs["Wq"], (1, 0, 2)).reshape(C, C)   # [c, (h hs)]
    Wk = np.transpose(inputs["Wk"], (1, 0, 2)).reshape(C, C)
    Wv = np.transpose(inputs["Wv"], (1, 0, 2)).reshape(C, C)
    Wo, W1, W2 = inputs["Wo"], inputs["W1"], inputs["W2"]
    f8 = lambda a: np.ascontiguousarray(a).astype(NPF8)
    z = np.zeros((128, C), np.float32)
    zf = np.zeros((128, DFF), np.float32)
    # pass A pairs K-rows (p, 256+p); pass B pairs (128+p, zero-pad)
    out = {
        "wqk_A": f8(np.concatenate(
            [Wq[0:128], Wk[0:128], Wq[256:384], Wk[256:384]], axis=1)),
        "wqk_B": f8(np.concatenate(
            [Wq[128:256], Wk[128:256], z, z], axis=1)),
        "wv_A": f8(np.concatenate([Wv[0:128], Wv[256:384]], axis=1)),
        "wv_B": f8(np.concatenate([Wv[128:256], z], axis=1)),
        "wo_t": np.ascontiguousarray(np.concatenate(
            [Wo[kt * 128:(kt + 1) * 128] for kt in range(KC)],
            axis=1)).astype(NPBF16),
        "w1_A": f8(np.concatenate([W1[0:128], W1[256:384]], axis=1)),
        "w1_B": f8(np.concatenate([W1[128:256], zf], axis=1)),
        "w2_dr": f8(np.concatenate(
            [np.concatenate([W2[mp * 256:mp * 256 + 128],
                             W2[mp * 256 + 128:mp * 256 + 256]], axis=1)
             for mp in range(KF // 2)], axis=1)),
    }
    return out


def make_in_maps(inputs, needed):
    prep = _prep_weights(inputs)
    in_maps = []
    for c in range(N_CORES):
        m = {}
        for nm in needed:
            if nm == "x":
                m[nm] = inputs["x"][c * BL:(c + 1) * BL]
            elif nm in prep:
                m[nm] = prep[nm]
            else:
                m[nm] = inputs[nm]
        in_maps.append(m)
    return in_maps


def needed_inputs(nc):
    needed = set()
    for alloc in nc.m.functions[0].allocations:
        if isinstance(alloc, mybir.MemoryLocationSet) and alloc.kind == "ExternalInput":
            nm = alloc.memorylocations[0].name
            if nm != "partition_id":
                needed.add(nm)
    return needed


def kernel(**inputs):
    inputs = {k: np.ascontiguousarray(np.asarray(v, dtype=np.float32))
              for k, v in inputs.items()}
    flags = _flags(inputs)
    key = tuple(sorted(flags.items()))
    nc = build(key, flags)
    in_maps = make_in_maps(inputs, needed_inputs(nc))
    res = run_bass_kernel_spmd(nc, in_maps, core_ids=list(range(N_CORES)))
    out = np.concatenate([res.results[c]["out"] for c in range(N_CORES)], axis=0)
    return out
